# revision 9
# baseline (speedup 1.0000x reference)
"""Trainium2 Bass kernel for InterpretableMultiHeadAttention.

Sharding: 8 cores = 4 batches x 2 query-row-parity groups (even/odd global
rows). Each core handles ONE batch and HALF of the query rows (interleaved),
computing ALL 8 heads locally -> the head-mean of the attention weights never
crosses cores, so there are no collectives. The even/odd interleave makes the
causal m-extent pattern identical on every core (single SPMD program); the
+-1 row difference in the diagonal mask is shipped as a host-precomputed
additive mask input.

All matmul contractions put the contracted dim on SBUF partitions, so the
host pre-transposes q/k/v (d-major) and pre-packs everything into
[128, kblock, free] layout for fully-contiguous DMA.

Pipeline per core:
  S1: qpT/kpT (e-major) and vp (m-major) projections, bf16 operands,
      fp32 PSUM, scaling folded into Wq/bq on host.
  S2: per 128-row l-tile: per head: scores matmul -> +mask (identity matmul)
      -> ACT Exp with fused row-sum (accum_out) -> r = 1/(8*Z) ->
      diag(r) matmul accumulates head-mean w directly in PSUM across heads.
      Evacuate w (fp32 -> output DMA; bf16 copy -> PE-transpose -> wT).
  S3: attnT[d',l] = sum_m vp[m,d'] * wT[m,l]  (PSUM accumulation over m).
  S4: out[l,:] = attnT.T @ Wo + bo (bias via rank-1 ones matmul into PSUM).
"""

import os
from contextlib import ExitStack

import numpy as np
import ml_dtypes

import concourse.bass as bass
import concourse.mybir as mybir
from concourse import bacc
from concourse.tile import TileContext
from concourse.bass_utils import run_bass_kernel_spmd

BF16 = mybir.dt.bfloat16
FP32 = mybir.dt.float32
AF = mybir.ActivationFunctionType
NPBF16 = ml_dtypes.bfloat16

D = 1024          # embed dim
H = 8             # heads
E = 128           # head dim
M = 2048          # key length
LLOC = 1024       # local query rows per core
NT = 8            # l-tiles of 128 local rows
NCH = [1, 1, 2, 2, 3, 3, 4, 4]   # m-chunks (512 wide) per l-tile
SCALING = E ** -0.5
NEG = -1.0e9
N_CORES = 8


def build_nc(finalize=True):
    nc = bacc.Bacc()

    qT = nc.dram_tensor("qT", [128, 8, LLOC], BF16, kind="ExternalInput")
    kT = nc.dram_tensor("kT", [128, 8, M], BF16, kind="ExternalInput")
    vT = nc.dram_tensor("vT", [128, 8, M], BF16, kind="ExternalInput")
    Wq = nc.dram_tensor("Wq", [128, 8, D], BF16, kind="ExternalInput")
    Wk = nc.dram_tensor("Wk", [128, 8, D], BF16, kind="ExternalInput")
    Wv = nc.dram_tensor("Wv", [128, 8, D], BF16, kind="ExternalInput")
    Wo = nc.dram_tensor("Wo", [128, 8, D], BF16, kind="ExternalInput")
    bqT = nc.dram_tensor("bqT", [E, H], FP32, kind="ExternalInput")
    bkT = nc.dram_tensor("bkT", [E, H], FP32, kind="ExternalInput")
    bv = nc.dram_tensor("bv", [1, D], BF16, kind="ExternalInput")
    bo = nc.dram_tensor("bo", [1, D], BF16, kind="ExternalInput")
    maskin = nc.dram_tensor("maskin", [128, NT, 512], BF16, kind="ExternalInput")
    ident = nc.dram_tensor("ident", [128, 128], BF16, kind="ExternalInput")
    onesin = nc.dram_tensor("onesin", [1, 128], BF16, kind="ExternalInput")

    w_out = nc.dram_tensor("w_out", [LLOC, M], FP32, kind="ExternalOutput")
    o_out = nc.dram_tensor("o_out", [LLOC, D], FP32, kind="ExternalOutput")

    with ExitStack() as ctx:
        tc = ctx.enter_context(TileContext(nc))
        consts = ctx.enter_context(tc.tile_pool(name="consts", bufs=1))
        persist = ctx.enter_context(tc.tile_pool(name="persist", bufs=1))

        I_sb = consts.tile([128, 128], BF16)
        nc.sync.dma_start(out=I_sb, in_=ident[:, :])
        ones_sb = consts.tile([1, 128], BF16)
        nc.sync.dma_start(out=ones_sb, in_=onesin[:, :])
        bq_sb = consts.tile([E, H], FP32)
        nc.sync.dma_start(out=bq_sb, in_=bqT[:, :])
        bk_sb = consts.tile([E, H], FP32)
        nc.sync.dma_start(out=bk_sb, in_=bkT[:, :])
        bv_sb = consts.tile([1, D], BF16)
        nc.sync.dma_start(out=bv_sb, in_=bv[:, :])
        bo_sb = consts.tile([1, D], BF16)
        nc.sync.dma_start(out=bo_sb, in_=bo[:, :])
        mask_sb = consts.tile([128, NT, 512], BF16)
        nc.sync.dma_start(out=mask_sb, in_=maskin[:, :, :])
        zero_sb = consts.tile([128, 512], FP32)
        nc.vector.memset(zero_sb, 0.0)

        qpT_sb = persist.tile([128, H, LLOC], BF16)      # [e, h, l]
        kpT_sb = persist.tile([128, H, M], BF16)         # [e, h, m]
        vp_sb = persist.tile([128, M // 128, D], BF16)   # [m%128, mb, d']
        wT_sb = persist.tile([128, M // 128, LLOC], BF16)  # [m%128, mb, l]
        nc.vector.memset(wT_sb, 0.0)

        # ---------------- Stage 1: projections ----------------
        with tc.tile_pool(name="xin", bufs=1) as xin, \
             tc.tile_pool(name="win", bufs=1) as win, \
             tc.tile_pool(name="ps1", bufs=4, space="PSUM") as ps1:

            # kpT[e,h,m] = (Wk.T @ k.T) + bk   (per head block)
            kT_sb = xin.tile([128, 8, M], BF16, tag="x", padded_shape=[128, 8, M])
            nc.sync.dma_start(out=kT_sb, in_=kT[:, :, :])
            Wk_sb = win.tile([128, 8, D], BF16, tag="w")
            nc.sync.dma_start(out=Wk_sb, in_=Wk[:, :, :])
            for h in range(H):
                for mc in range(M // 512):
                    ps = ps1.tile([128, 512], FP32, tag="p1")
                    for kb in range(8):
                        nc.tensor.matmul(
                            ps,
                            lhsT=Wk_sb[:, kb, h * 128:(h + 1) * 128],
                            rhs=kT_sb[:, kb, mc * 512:(mc + 1) * 512],
                            start=(kb == 0), stop=(kb == 7))
                    nc.scalar.activation(
                        out=kpT_sb[:, h, mc * 512:(mc + 1) * 512], in_=ps,
                        func=AF.Identity, bias=bk_sb[:, h:h + 1])

            # qpT[e,h,l] (scaling folded into Wq/bq on host)
            qT_sb = xin.tile([128, 8, LLOC], BF16, tag="xq")
            nc.sync.dma_start(out=qT_sb, in_=qT[:, :, :])
            Wq_sb = win.tile([128, 8, D], BF16, tag="w")
            nc.sync.dma_start(out=Wq_sb, in_=Wq[:, :, :])
            for h in range(H):
                for lc in range(LLOC // 512):
                    ps = ps1.tile([128, 512], FP32, tag="p1")
                    for kb in range(8):
                        nc.tensor.matmul(
                            ps,
                            lhsT=Wq_sb[:, kb, h * 128:(h + 1) * 128],
                            rhs=qT_sb[:, kb, lc * 512:(lc + 1) * 512],
                            start=(kb == 0), stop=(kb == 7))
                    nc.scalar.activation(
                        out=qpT_sb[:, h, lc * 512:(lc + 1) * 512], in_=ps,
                        func=AF.Identity, bias=bq_sb[:, h:h + 1])

            # vp[m, d'] natural layout; bias via rank-1 ones x bv matmul
            vT_sb = xin.tile([128, 8, M], BF16, tag="x", padded_shape=[128, 8, M])
            nc.sync.dma_start(out=vT_sb, in_=vT[:, :, :])
            Wv_sb = win.tile([128, 8, D], BF16, tag="w")
            nc.sync.dma_start(out=Wv_sb, in_=Wv[:, :, :])
            for mb in range(M // 128):
                for dc in range(D // 512):
                    ps = ps1.tile([128, 512], FP32, tag="p1")
                    for kb in range(8):
                        nc.tensor.matmul(
                            ps,
                            lhsT=vT_sb[:, kb, mb * 128:(mb + 1) * 128],
                            rhs=Wv_sb[:, kb, dc * 512:(dc + 1) * 512],
                            start=(kb == 0), stop=False)
                    nc.tensor.matmul(
                        ps, lhsT=ones_sb[0:1, :],
                        rhs=bv_sb[0:1, dc * 512:(dc + 1) * 512],
                        start=False, stop=True)
                    nc.scalar.activation(
                        out=vp_sb[:, mb, dc * 512:(dc + 1) * 512], in_=ps,
                        func=AF.Copy)

        # ---------------- Stage 2: scores/softmax/head-mean ----------------
        with tc.tile_pool(name="s2", bufs=2) as s2p, \
             tc.tile_pool(name="wm", bufs=2) as wmp, \
             tc.tile_pool(name="wmps", bufs=4, space="PSUM") as wmps, \
             tc.tile_pool(name="scps", bufs=2, space="PSUM") as scps, \
             tc.tile_pool(name="tpps", bufs=2, space="PSUM") as tpps:
            for t in range(NT):
                nch = NCH[t]
                wm_ps = []
                for c in range(nch):
                    wmt = wmps.tile([128, 512], FP32, tag="wm",
                                    name=f"wm_ps_{t}_{c}")
                    wm_ps.append(wmt)
                for h in range(H):
                    exp_sb = s2p.tile([128, 4, 512], BF16, tag="exp")
                    z_sb = s2p.tile([128, 4], FP32, tag="z")
                    for c in range(nch):
                        sc = scps.tile([128, 512], FP32, tag="sc",
                                       name=f"sc_{t}_{h}_{c}")
                        last = (c == nch - 1)
                        nc.tensor.matmul(
                            sc,
                            lhsT=qpT_sb[:, h, t * 128:(t + 1) * 128],
                            rhs=kpT_sb[:, h, c * 512:(c + 1) * 512],
                            start=True, stop=not last)
                        if last:
                            nc.tensor.matmul(
                                sc, lhsT=I_sb, rhs=mask_sb[:, t, :],
                                start=False, stop=True)
                        nc.scalar.activation(
                            out=exp_sb[:, c, :], in_=sc, func=AF.Exp,
                            accum_out=z_sb[:, c:c + 1])
                    zs = s2p.tile([128, 1], FP32, tag="zs")
                    nc.vector.tensor_reduce(
                        out=zs, in_=z_sb[:, 0:nch], axis=mybir.AxisListType.X,
                        op=mybir.AluOpType.add)
                    rz = s2p.tile([128, 1], FP32, tag="rz")
                    nc.vector.reciprocal(rz, zs)
                    r8 = s2p.tile([128, 1], FP32, tag="r8")
                    nc.vector.tensor_scalar_mul(r8, rz, 0.125)
                    diag = s2p.tile([128, 128], BF16, tag="diag")
                    nc.vector.tensor_scalar_mul(diag, I_sb, r8)
                    for c in range(nch):
                        nc.tensor.matmul(
                            wm_ps[c], lhsT=diag, rhs=exp_sb[:, c, :],
                            start=(h == 0), stop=(h == H - 1),
                            skip_group_check=True)
                # evacuate w_mean; DMA out; transpose into wT
                wm_f = wmp.tile([128, 4, 512], FP32, tag="wmf")
                wm_b = wmp.tile([128, 4, 512], BF16, tag="wmb")
                for c in range(nch):
                    nc.scalar.activation(out=wm_f[:, c, :], in_=wm_ps[c],
                                         func=AF.Copy)
                    nc.vector.tensor_copy(out=wm_b[:, c, :], in_=wm_ps[c])
                nc.sync.dma_start(
                    out=w_out[t * 128:(t + 1) * 128, 0:nch * 512],
                    in_=wm_f[:, 0:nch, :])
                for zc in range(nch, 4):
                    nc.sync.dma_start(
                        out=w_out[t * 128:(t + 1) * 128, zc * 512:(zc + 1) * 512],
                        in_=zero_sb)
                for mb in range(2 * t + 2):
                    tp = tpps.tile([128, 128], BF16, tag="tp",
                                   name=f"tp_{t}_{mb}")
                    nc.tensor.transpose(
                        tp, wm_b[:, mb // 4, (mb % 4) * 128:(mb % 4 + 1) * 128],
                        I_sb)
                    nc.scalar.activation(
                        out=wT_sb[:, mb, t * 128:(t + 1) * 128], in_=tp,
                        func=AF.Copy)

        # ------- Stage 3: attnT = vp.T-contracted with wT; Stage 4: out -----
        with tc.tile_pool(name="s34", bufs=1) as s34, \
             tc.tile_pool(name="os", bufs=2) as osp, \
             tc.tile_pool(name="ps3", bufs=4, space="PSUM") as ps3, \
             tc.tile_pool(name="ps4", bufs=4, space="PSUM") as ps4:
            attnT_sb = s34.tile([128, H, LLOC], BF16)    # [d'%128, db, l]
            for lc in range(LLOC // 512):
                nmb = 8 * lc + 8
                for db in range(H):
                    ap = ps3.tile([128, 512], FP32, tag="at")
                    for mb in range(nmb):
                        nc.tensor.matmul(
                            ap,
                            lhsT=vp_sb[:, mb, db * 128:(db + 1) * 128],
                            rhs=wT_sb[:, mb, lc * 512:(lc + 1) * 512],
                            start=(mb == 0), stop=(mb == nmb - 1))
                    nc.scalar.activation(
                        out=attnT_sb[:, db, lc * 512:(lc + 1) * 512], in_=ap,
                        func=AF.Copy)

            # ---------------- Stage 4: out projection ----------------
            Wo_sb = s34.tile([128, 8, D], BF16, tag="w")
            nc.sync.dma_start(out=Wo_sb, in_=Wo[:, :, :])
            for t in range(NT):
                o_sb = osp.tile([128, D], FP32, tag="o")
                for dc in range(D // 512):
                    op = ps4.tile([128, 512], FP32, tag="op")
                    for db in range(H):
                        nc.tensor.matmul(
                            op,
                            lhsT=attnT_sb[:, db, t * 128:(t + 1) * 128],
                            rhs=Wo_sb[:, db, dc * 512:(dc + 1) * 512],
                            start=(db == 0), stop=False)
                    nc.tensor.matmul(
                        op, lhsT=ones_sb[0:1, :],
                        rhs=bo_sb[0:1, dc * 512:(dc + 1) * 512],
                        start=False, stop=True)
                    nc.scalar.activation(
                        out=o_sb[:, dc * 512:(dc + 1) * 512], in_=op,
                        func=AF.Copy)
                nc.sync.dma_start(out=o_out[t * 128:(t + 1) * 128, :], in_=o_sb)

    if finalize:
        nc.finalize()
    return nc


def _pack_w(W):
    # [D, D] -> [128, 8, D] with [p, kb, n] = W[kb*128+p, n]
    return np.ascontiguousarray(
        W.reshape(8, 128, D).transpose(1, 0, 2)).astype(NPBF16)


def _pack_xT(x2d, ncols):
    # x2d: [rows, D] -> xT packed [128, 8, rows] with [p, kb, r] = x2d[r, kb*128+p]
    xt = np.ascontiguousarray(x2d.T)  # [D, rows]
    return np.ascontiguousarray(
        xt.reshape(8, 128, ncols).transpose(1, 0, 2)).astype(NPBF16)


def make_in_maps(q, k, v, Wq, bq, Wk, bk, Wv, bv, Wo, bo):
    q = np.asarray(q, np.float32)
    k = np.asarray(k, np.float32)
    v = np.asarray(v, np.float32)
    Wq_e = np.asarray(Wq, np.float32) * SCALING
    bq_e = np.asarray(bq, np.float32) * SCALING
    Wq_p = _pack_w(Wq_e)
    Wk_p = _pack_w(np.asarray(Wk, np.float32))
    Wv_p = _pack_w(np.asarray(Wv, np.float32))
    Wo_p = _pack_w(np.asarray(Wo, np.float32))
    bqT = np.ascontiguousarray(bq_e.reshape(8, 128).T).astype(np.float32)
    bkT = np.ascontiguousarray(
        np.asarray(bk, np.float32).reshape(8, 128).T).astype(np.float32)
    bv_r = np.asarray(bv, np.float32).reshape(1, D).astype(NPBF16)
    bo_r = np.asarray(bo, np.float32).reshape(1, D).astype(NPBF16)
    ident = np.eye(128, dtype=NPBF16)
    onesr = np.ones((1, 128), dtype=NPBF16)

    in_maps = []
    for c in range(N_CORES):
        b, par = divmod(c, 2)
        qs = q[par::2, b, :]                    # [1024, D]
        ks = k[:, b, :]                         # [2048, D]
        vs = v[:, b, :]
        # additive mask for the last (diagonal) 512-chunk of each l-tile
        mask = np.zeros((128, NT, 512), np.float32)
        p_idx = np.arange(128)
        for t in range(NT):
            lg = 256 * t + 2 * p_idx + par      # global row per partition
            mcol = (NCH[t] - 1) * 512 + np.arange(512)
            mask[:, t, :] = np.where(mcol[None, :] <= lg[:, None], 0.0, NEG)
        in_maps.append({
            "qT": _pack_xT(qs, LLOC),
            "kT": _pack_xT(ks, M),
            "vT": _pack_xT(vs, M),
            "Wq": Wq_p, "Wk": Wk_p, "Wv": Wv_p, "Wo": Wo_p,
            "bqT": bqT, "bkT": bkT, "bv": bv_r, "bo": bo_r,
            "maskin": mask.astype(NPBF16),
            "ident": ident, "onesin": onesr,
        })
    return in_maps


_CACHE = {}


def kernel(q, k, v, Wq, bq, Wk, bk, Wv, bv, Wo, bo):
    if "nc" not in _CACHE:
        _CACHE["nc"] = build_nc()
    nc = _CACHE["nc"]
    in_maps = make_in_maps(q, k, v, Wq, bq, Wk, bk, Wv, bv, Wo, bo)
    trace = os.environ.get("KTRACE", "0") == "1"
    rb = run_bass_kernel_spmd(
        nc, in_maps, core_ids=list(range(N_CORES)), trace=trace)
    kernel.last_results = rb

    out = np.zeros((2048, 4, D), np.float32)
    w_mean = np.zeros((4, 2048, 2048), np.float32)
    for c in range(N_CORES):
        b, par = divmod(c, 2)
        res = rb.results[c]
        w_mean[b, par::2, :] = res["w_out"]
        out[par::2, b, :] = res["o_out"]
    return out, w_mean
